# revision 22
# baseline (speedup 1.0000x reference)
"""DiscreteBipartiteFlow forward on 8 trn2 NeuronCores.

Math: inputs rows are exact one-hots (x0|x1). net = relu(x0@W1+b1)@W2+b2
only depends on i0=argmax(x0), so precompute (on device, per core) the
[V, 2V] table NET = relu(W1+b1)@W2+b2 and its per-row argmaxes
L[i]=argmax(NET[i,:V]), S[i]=argmax(NET[i,V:]). The straight-through
one_hot_argmax is numerically exactly-hard (off-argmax entries cancel to
exactly 0.0 in fp32), one_hot_multiply of a one-hot x1 by the one-hot
scale is an index product, and one_hot_add is an index sum, so
z1 = one_hot((L[i0] + a1*S[i0]) mod V) (or all-zeros when S[i0]==0,
since scale index 0 is excluded). Output = [x0 | z1].

Schedule (per core, 1024 rows, 8 rows per partition):
 - ONE packed weights DMA (w1t|w2|iota|identity host-marshalled) + two x
   half DMAs on the sync queue; x0 passthrough streams back out on the
   same queue. Few large DMAs: each HWDGE dispatch is ~0.7us serial.
 - Table: relu on ACT, NET matmuls on PE, argmax directly on the NET
   PSUM via reduce_max + is_ge*iota + reduce_max on DVE (the
   MAX8/FIND_INDEX8 pair costs 3.4us; this chain is ~5x cheaper). When
   b1/b2 are nonzero (never for this model's init) a slower bias path
   is compiled in instead.
 - pack as TWO bf16 columns [L | S+128*[S==0]] so the gather matmuls run
   in 1-pass bf16 instead of 2-pass fp32.
 - Data side: a1 = sum(x1*iota) muls on GPSIMD, reduces on DVE; x0
   blocks PE-transposed (identity ships in wpack), ACT evacuates PSUM to
   bf16, 8 tiny bf16 PE matmuls gather [L|S'] [i0] into one PSUM tile.
 - int32 unpack on DVE (power-of-2 mod via &/>>), dead-flag (S==0) folds
   into an out-of-range compare index, one broadcast is_equal per half
   builds z1, z1 streams out on the scalar
   queue.
Data-parallel over 8 cores; weights replicated.
"""

import numpy as np

V = 128
H = 512
N_CORES = 8
P = 128
NJ = 8               # row slots per partition
NH = 2               # halves
HJ = NJ // NH        # slots per half

KH = H // P          # 4
# wpack float offsets (fast path, b1 == 0 and b2 == 0)
W1T_OFF = 0                       # [P, KH*V]  w1t[p, k*V+i] = W1[i, k*P+p]
W2_OFF = W1T_OFF + KH * V         # [P, KH*2V] w2p[p, k*2V+n] = W2[k*P+p, n]
IOTA_OFF = W2_OFF + KH * 2 * V    # [P, V]     iota replicated
IDENT_OFF = IOTA_OFF + V          # [P, V]     identity matrix
WPK = IDENT_OFF + V
# slow-path extras appended when biases are nonzero
B1_OFF = WPK                      # [P, KH]
B2_OFF = WPK + KH                 # [P, 2V]


def build_bass(rows: int, has_b1: bool, has_b2: bool):
    """Build the single-core Bass program for a [rows, 2V] batch shard."""
    import concourse.bacc as bacc
    import concourse.bass as bass
    import concourse.tile as tile
    from concourse import mybir

    f32 = mybir.dt.float32
    bf16 = mybir.dt.bfloat16
    i32 = mybir.dt.int32
    A = mybir.AluOpType
    AF = mybir.ActivationFunctionType
    X = mybir.AxisListType.X

    assert rows == P * NJ
    wpk = WPK + (KH if has_b1 else 0) + (2 * V if has_b2 else 0)

    nc = bacc.Bacc(None)
    x = nc.declare_dram_parameter("x", [rows, 2 * V], bf16, isOutput=False)
    wpack = nc.declare_dram_parameter("wpack", [P, wpk], f32, isOutput=False)
    out = nc.declare_dram_parameter("out", [rows, 2 * V], f32, isOutput=True)

    x_r = x.rearrange("(p j) n -> p j n", j=NJ)
    out_r = out.rearrange("(p j) n -> p j n", j=NJ)

    def bcast_mid(t_ap, reps):
        return bass.AP(
            tensor=t_ap.tensor, offset=t_ap.offset,
            ap=[t_ap.ap[0], [0, reps]] + list(t_ap.ap[1:]),
        )

    def bcast_last(t_ap, reps):
        return bass.AP(
            tensor=t_ap.tensor, offset=t_ap.offset,
            ap=list(t_ap.ap) + [[0, reps]],
        )

    def view3(t_ap, n, m):
        # [P, n*m] 2D AP -> [P, n, m] row-major view
        return bass.AP(
            tensor=t_ap.tensor, offset=t_ap.offset,
            ap=[t_ap.ap[0], [m, n], [1, m]],
        )

    def stridev(t_ap, start, step, n):
        # [P, K] 2D AP -> [P, n] view of columns start, start+step, ...
        return bass.AP(
            tensor=t_ap.tensor, offset=t_ap.offset + start,
            ap=[t_ap.ap[0], [step, n]],
        )

    with tile.TileContext(nc) as tc:
        with (
            tc.tile_pool(name="main", bufs=1) as main,
            tc.tile_pool(name="pnet", bufs=1, space="PSUM") as pnet,
            tc.tile_pool(name="pq", bufs=1, space="PSUM") as pq,
            tc.tile_pool(name="ptp", bufs=4, space="PSUM") as ptp,
        ):
            # ---- loads (sync queue) ----
            wp = main.tile([P, wpk], f32)
            nc.sync.dma_start(out=wp, in_=wpack[:, :])
            xh = []
            for h in range(NH):
                t = main.tile([P, HJ, 2 * V], bf16, tag=f"xh{h}")
                nc.sync.dma_start(out=t, in_=x_r[:, h * HJ : (h + 1) * HJ, :])
                xh.append(t)

            iota = wp[:, IOTA_OFF : IOTA_OFF + V]
            ident = wp[:, IDENT_OFF : IDENT_OFF + V]
            # bf16 iota (exact for 0..127), cast once on ACT
            iotb = main.tile([P, V], bf16)
            nc.scalar.activation(out=iotb, in_=iota, func=AF.Copy)

            # x0 passthrough: ACT casts bf16->f32, stores on sync queue
            xpass = []
            for h in range(NH):
                t = main.tile([P, HJ, V], f32, tag=f"xp{h}")
                nc.scalar.activation(out=t, in_=xh[h][:, :, 0:V], func=AF.Copy)
                xpass.append(t)
            for h in range(NH):
                nc.sync.dma_start(
                    out=out_r[:, h * HJ : (h + 1) * HJ, 0:V],
                    in_=xpass[h],
                )

            # ---- table: relu on ACT, NET on PE ----
            ht = main.tile([P, H], f32)
            for k in range(KH):
                nc.scalar.activation(
                    out=ht[:, k * V : (k + 1) * V],
                    in_=wp[:, W1T_OFF + k * V : W1T_OFF + (k + 1) * V],
                    func=AF.Relu,
                    bias=(wp[:, B1_OFF + k : B1_OFF + k + 1] if has_b1 else 0.0),
                    scale=1.0,
                )
            net_ps = pnet.tile([P, 2 * V], f32)
            for k in range(KH):
                nc.tensor.matmul(
                    net_ps,
                    lhsT=ht[:, k * V : (k + 1) * V],
                    rhs=wp[:, W2_OFF + k * 2 * V : W2_OFF + (k + 1) * 2 * V],
                    start=(k == 0),
                    stop=(k == KH - 1),
                )

            # ---- GPSIMD: a1 muls (x1 * iota), reduced later on DVE ----
            scs = []
            for h in range(NH):
                sc = main.tile([P, HJ, V], bf16, tag=f"sc{h}")
                nc.gpsimd.tensor_tensor(
                    out=sc, in0=xh[h][:, :, V : 2 * V], in1=bcast_mid(iotb, HJ),
                    op=A.mult,
                )
                scs.append(sc)

            # ---- DVE: a1 reduces + argmax on NET psum + bf16 pack ----
            a1f = main.tile([P, NJ], f32)
            nc.vector.reduce_sum(a1f[:, 0:HJ], scs[0], axis=X)
            nc.vector.reduce_sum(a1f[:, HJ:NJ], scs[1], axis=X)
            a1i = main.tile([P, NJ], i32)
            nc.vector.tensor_copy(a1i, a1f)
            if has_b2:
                netb = main.tile([P, 2 * V], f32)
                nc.vector.tensor_tensor(
                    out=netb, in0=net_ps,
                    in1=wp[:, B2_OFF : B2_OFF + 2 * V], op=A.add,
                )
                netsrc = netb
            else:
                netsrc = net_ps
            net3 = view3(netsrc, 2, V)
            mx2 = main.tile([P, 2], f32)
            nc.vector.reduce_max(mx2, net3, axis=X)
            # identC = ident + 0*net: data-equal to ident but gates the PE
            # transposes behind the NET matmuls so the scheduler cannot
            # hoist them ahead (costs ~1.9us of PE idle otherwise); bf16
            # out so the transposes run 1-pass
            z0 = main.tile([P, 1], f32)
            nc.vector.tensor_scalar(
                out=z0, in0=mx2[:, 0:1], scalar1=0.0, scalar2=None, op0=A.mult
            )
            identC = main.tile([P, V], bf16)
            nc.vector.tensor_scalar(
                out=identC, in0=ident, scalar1=z0[:, 0:1], scalar2=None, op0=A.add
            )
            ge2 = main.tile([P, 2, V], bf16)
            nc.vector.tensor_tensor(
                out=ge2, in0=net3, in1=bcast_last(mx2, V), op=A.is_ge
            )
            ix2 = main.tile([P, 2, V], bf16)
            nc.vector.tensor_tensor(
                out=ix2, in0=ge2, in1=bcast_mid(iotb, 2), op=A.mult
            )
            LS = main.tile([P, 2], f32)
            nc.vector.reduce_max(LS, ix2, axis=X)
            # pkf = [L | S + 128*[S==0]]  (both <= 255, exact in bf16)
            zinv = main.tile([P, 1], f32)
            nc.vector.tensor_scalar(
                out=zinv, in0=LS[:, 1:2], scalar1=0.5, scalar2=None, op0=A.is_lt
            )
            pkb = main.tile([P, 2], bf16)
            nc.vector.tensor_copy(pkb[:, 0:1], LS[:, 0:1])
            nc.vector.tensor_scalar(
                out=pkb[:, 1:2], in0=zinv, scalar1=float(V), scalar2=LS[:, 1:2],
                op0=A.mult, op1=A.add,
            )

            # ---- PE transposes + ACT bf16 evac + PE [L|S'] gather ----
            xTs = []
            for h in range(NH):
                for j in range(HJ):
                    tp = ptp.tile([P, P], bf16, tag="tp", bufs=4)
                    nc.tensor.transpose(tp, xh[h][:, j, 0:V], identC)
                    xT = main.tile([P, P], bf16, tag=f"xT{h}{j}")
                    nc.scalar.activation(out=xT, in_=tp, func=AF.Copy)
                    xTs.append(xT)
            qhl = pq.tile([P, 2 * NJ], f32)
            for sj in range(NJ):
                nc.tensor.matmul(
                    qhl[:, 2 * sj : 2 * sj + 2], lhsT=xTs[sj], rhs=pkb,
                    start=True, stop=True,
                )

            # ---- DVE: unpack -> c = (L + a1*S) & 127 | dead-flag ----
            qi = main.tile([P, 2 * NJ], i32)
            nc.vector.tensor_copy(qi, qhl)
            lov = stridev(qi, 0, 2, NJ)
            hiv = stridev(qi, 1, 2, NJ)
            si = main.tile([P, NJ], i32)
            nc.vector.tensor_scalar(out=si, in0=hiv, scalar1=V - 1, scalar2=None, op0=A.bitwise_and)
            di = main.tile([P, NJ], i32)
            nc.vector.tensor_scalar(out=di, in0=hiv, scalar1=V, scalar2=None, op0=A.bitwise_and)
            ti = main.tile([P, NJ], i32)
            nc.vector.tensor_mul(ti, si, a1i)
            nc.vector.tensor_tensor(out=ti, in0=ti, in1=lov, op=A.add)
            ci = main.tile([P, NJ], i32)
            nc.vector.tensor_scalar(out=ci, in0=ti, scalar1=V - 1, scalar2=None, op0=A.bitwise_and)
            nc.vector.tensor_tensor(out=ci, in0=ci, in1=di, op=A.bitwise_or)
            cf = main.tile([P, NJ], f32)
            nc.vector.tensor_copy(cf, ci)

            # ---- z1 build (h0 on DVE, h1 on GPSIMD) + store (scalar q) ----
            for h in range(NH):
                zt = main.tile([P, HJ, V], f32, tag=f"zt{h}")
                nc.vector.tensor_tensor(
                    out=zt,
                    in0=bcast_mid(iota, HJ),
                    in1=bcast_last(cf[:, h * HJ : (h + 1) * HJ], V),
                    op=A.is_equal,
                )
                seng = nc.sync if h == 0 else nc.scalar
                seng.dma_start(
                    out=out_r[:, h * HJ : (h + 1) * HJ, V : 2 * V], in_=zt
                )

    nc.finalize()
    return nc


def _pack_weights(W1, b1, W2, b2, has_b1, has_b2) -> np.ndarray:
    """Pure layout marshalling of the MLP weights into one [P, wpk] block."""
    wpk = WPK + (KH if has_b1 else 0) + (2 * V if has_b2 else 0)
    wpack = np.empty((P, wpk), np.float32)
    # w1t[p, k*V+i] = W1[i, k*P+p]
    wpack[:, W1T_OFF:W2_OFF] = (
        W1.T.reshape(KH, P, V).transpose(1, 0, 2).reshape(P, KH * V)
    )
    # w2p[p, k*2V+n] = W2[k*P+p, n]
    wpack[:, W2_OFF:IOTA_OFF] = (
        W2.reshape(KH, P, 2 * V).transpose(1, 0, 2).reshape(P, KH * 2 * V)
    )
    wpack[:, IOTA_OFF:IDENT_OFF] = np.arange(V, dtype=np.float32)
    wpack[:, IDENT_OFF:WPK] = np.eye(V, dtype=np.float32)
    off = WPK
    if has_b1:
        wpack[:, off : off + KH] = b1.reshape(KH, P).T
        off += KH
    if has_b2:
        wpack[:, off : off + 2 * V] = b2.reshape(1, 2 * V)
    return wpack


# Test-harness hooks: extra kwargs for run_bass_kernel_spmd (e.g. trace=True)
# and the last BassKernelResults for profiling. Unused when graded.
RUN_KWARGS: dict = {}
LAST_RESULTS = None


def kernel(**inputs) -> np.ndarray:
    global LAST_RESULTS
    import ml_dtypes
    from concourse.bass_utils import run_bass_kernel_spmd

    # lossless cast: inputs are exact one-hots (0.0/1.0), both exact in bf16
    x = np.ascontiguousarray(
        np.asarray(inputs["inputs"], dtype=np.float32).astype(ml_dtypes.bfloat16)
    )
    W1 = np.asarray(inputs["W1"], dtype=np.float32)
    b1 = np.asarray(inputs["b1"], dtype=np.float32)
    W2 = np.asarray(inputs["W2"], dtype=np.float32)
    b2 = np.asarray(inputs["b2"], dtype=np.float32)
    has_b1 = bool(np.any(b1))
    has_b2 = bool(np.any(b2))
    wpack = _pack_weights(W1, b1, W2, b2, has_b1, has_b2)

    B = x.shape[0]
    rows = B // N_CORES
    nc = build_bass(rows, has_b1, has_b2)

    shards = np.split(x, N_CORES, axis=0)
    in_maps = [{"x": s, "wpack": wpack} for s in shards]
    res = run_bass_kernel_spmd(nc, in_maps, list(range(N_CORES)), **RUN_KWARGS)
    LAST_RESULTS = res
    return np.concatenate([r["out"] for r in res.results], axis=0)


# revision 23
# speedup vs baseline: 1.0875x; 1.0875x over previous
"""DiscreteBipartiteFlow forward on 8 trn2 NeuronCores.

Math: inputs rows are exact one-hots (x0|x1). net = relu(x0@W1+b1)@W2+b2
only depends on i0=argmax(x0), so precompute (on device, per core) the
[V, 2V] table NET = relu(W1+b1)@W2+b2 and its per-row argmaxes
L[i]=argmax(NET[i,:V]), S[i]=argmax(NET[i,V:]). The straight-through
one_hot_argmax is numerically exactly-hard (off-argmax entries cancel to
exactly 0.0 in fp32), one_hot_multiply of a one-hot x1 by the one-hot
scale is an index product, and one_hot_add is an index sum, so
z1 = one_hot((L[i0] + a1*S[i0]) mod V) (or all-zeros when S[i0]==0,
since scale index 0 is excluded). Output = [x0 | z1].

Schedule (per core, 1024 rows, 8 rows per partition):
 - x ships as bf16 (lossless for one-hots) halving its DMA; weights as
   two packed DMAs (w1t first so the ACT relus start ~3us earlier, then
   w2|iota|identity); x0 passthrough is ACT-cast back to f32 and
   streamed out on the sync queue.
 - Table: relu on ACT first-in-queue, NET matmuls on PE, argmax directly
   on the NET PSUM via reduce_max + is_ge*iota + reduce_max on DVE.
   identC = ident + 0*NET gates the PE transposes behind the NET
   matmuls so the tile scheduler cannot hoist them ahead. When b1/b2
   are nonzero (never for this model's init) a slower bias path is
   compiled in.
 - a1 = sum(x1*iota) bf16 mul+reduce per half on DVE, completing inside
   the NET window; pack as TWO bf16 columns [L | S+128*[S==0]] so the
   gather matmuls run 1-pass bf16 into one PSUM tile.
 - int32 unpack on DVE (power-of-2 mod via &/>>), dead-flag (S==0)
   folds into an out-of-range compare index, ONE bf16-compare is_equal
   builds all of z1, halves stream out on sync+scalar queues in
   parallel.
Data-parallel over 8 cores; weights replicated.
"""

import numpy as np

V = 128
H = 512
N_CORES = 8
P = 128
NJ = 8               # row slots per partition
NH = 2               # halves
HJ = NJ // NH        # slots per half

KH = H // P          # 4
# wpack float offsets (fast path, b1 == 0 and b2 == 0)
W1T_OFF = 0                       # [P, KH*V]  w1t[p, k*V+i] = W1[i, k*P+p]
W2_OFF = W1T_OFF + KH * V         # [P, KH*2V] w2p[p, k*2V+n] = W2[k*P+p, n]
IOTA_OFF = W2_OFF + KH * 2 * V    # [P, V]     iota replicated
IDENT_OFF = IOTA_OFF + V          # [P, V]     identity matrix
WPK = IDENT_OFF + V
# slow-path extras appended when biases are nonzero
B1_OFF = WPK                      # [P, KH]
B2_OFF = WPK + KH                 # [P, 2V]


def build_bass(rows: int, has_b1: bool, has_b2: bool):
    """Build the single-core Bass program for a [rows, 2V] batch shard."""
    import concourse.bacc as bacc
    import concourse.bass as bass
    import concourse.tile as tile
    from concourse import mybir

    f32 = mybir.dt.float32
    bf16 = mybir.dt.bfloat16
    i32 = mybir.dt.int32
    A = mybir.AluOpType
    AF = mybir.ActivationFunctionType
    X = mybir.AxisListType.X

    assert rows == P * NJ
    wpk = WPK + (KH if has_b1 else 0) + (2 * V if has_b2 else 0)

    nc = bacc.Bacc(None)
    x = nc.declare_dram_parameter("x", [rows, 2 * V], bf16, isOutput=False)
    wpack = nc.declare_dram_parameter("wpack", [P, wpk], f32, isOutput=False)
    out = nc.declare_dram_parameter("out", [rows, 2 * V], f32, isOutput=True)

    x_r = x.rearrange("(p j) n -> p j n", j=NJ)
    out_r = out.rearrange("(p j) n -> p j n", j=NJ)

    def bcast_mid(t_ap, reps):
        return bass.AP(
            tensor=t_ap.tensor, offset=t_ap.offset,
            ap=[t_ap.ap[0], [0, reps]] + list(t_ap.ap[1:]),
        )

    def bcast_last(t_ap, reps):
        return bass.AP(
            tensor=t_ap.tensor, offset=t_ap.offset,
            ap=list(t_ap.ap) + [[0, reps]],
        )

    def view3(t_ap, n, m):
        # [P, n*m] 2D AP -> [P, n, m] row-major view
        return bass.AP(
            tensor=t_ap.tensor, offset=t_ap.offset,
            ap=[t_ap.ap[0], [m, n], [1, m]],
        )

    def stridev(t_ap, start, step, n):
        # [P, K] 2D AP -> [P, n] view of columns start, start+step, ...
        return bass.AP(
            tensor=t_ap.tensor, offset=t_ap.offset + start,
            ap=[t_ap.ap[0], [step, n]],
        )

    with tile.TileContext(nc) as tc:
        with (
            tc.tile_pool(name="main", bufs=1) as main,
            tc.tile_pool(name="pnet", bufs=1, space="PSUM") as pnet,
            tc.tile_pool(name="pq", bufs=1, space="PSUM") as pq,
            tc.tile_pool(name="ptp", bufs=4, space="PSUM") as ptp,
        ):
            # ---- loads (sync queue): w1t first so relu starts early ----
            wpa = main.tile([P, KH * V], f32)
            nc.sync.dma_start(out=wpa, in_=wpack[:, 0 : KH * V])
            wpb = main.tile([P, wpk - KH * V], f32)
            nc.sync.dma_start(out=wpb, in_=wpack[:, KH * V : wpk])
            xh = []
            for h in range(NH):
                t = main.tile([P, HJ, 2 * V], bf16, tag=f"xh{h}")
                nc.sync.dma_start(out=t, in_=x_r[:, h * HJ : (h + 1) * HJ, :])
                xh.append(t)

            def wb(lo, hi):
                return wpb[:, lo - KH * V : hi - KH * V]

            iota = wb(IOTA_OFF, IOTA_OFF + V)
            ident = wb(IDENT_OFF, IDENT_OFF + V)

            # ---- ACT: relus first-in-queue, then passthrough casts ----
            ht = main.tile([P, H], f32)
            for k in range(KH):
                nc.scalar.activation(
                    out=ht[:, k * V : (k + 1) * V],
                    in_=wpa[:, k * V : (k + 1) * V],
                    func=AF.Relu,
                    bias=(wb(B1_OFF + k, B1_OFF + k + 1) if has_b1 else 0.0),
                    scale=1.0,
                )
            xpass = []
            for h in range(NH):
                t = main.tile([P, HJ, V], f32, tag=f"xp{h}")
                nc.scalar.activation(out=t, in_=xh[h][:, :, 0:V], func=AF.Copy)
                xpass.append(t)
            for h in range(NH):
                nc.sync.dma_start(
                    out=out_r[:, h * HJ : (h + 1) * HJ, 0:V],
                    in_=xpass[h],
                )

            # ---- PE: NET matmuls ----
            net_ps = pnet.tile([P, 2 * V], f32)
            for k in range(KH):
                nc.tensor.matmul(
                    net_ps,
                    lhsT=ht[:, k * V : (k + 1) * V],
                    rhs=wb(W2_OFF + k * 2 * V, W2_OFF + (k + 1) * 2 * V),
                    start=(k == 0),
                    stop=(k == KH - 1),
                )

            # ---- DVE: bf16 iota, a1 dots (inside the NET window) ----
            iotb = main.tile([P, V], bf16)
            nc.vector.tensor_copy(iotb, iota)
            a1f = main.tile([P, NJ], f32)
            for h in range(NH):
                sc = main.tile([P, HJ, V], bf16, tag=f"sc{h}")
                nc.vector.tensor_tensor(
                    out=sc, in0=xh[h][:, :, V : 2 * V],
                    in1=bcast_mid(iotb, HJ), op=A.mult,
                )
                nc.vector.reduce_sum(a1f[:, h * HJ : (h + 1) * HJ], sc, axis=X)
            a1i = main.tile([P, NJ], i32)
            nc.vector.tensor_copy(a1i, a1f)

            # ---- DVE: identC (NET-gated ident), argmax, bf16 pack ----
            if has_b2:
                netb = main.tile([P, 2 * V], f32)
                nc.vector.tensor_tensor(
                    out=netb, in0=net_ps,
                    in1=wb(B2_OFF, B2_OFF + 2 * V), op=A.add,
                )
                netsrc = netb
            else:
                netsrc = net_ps
            # identC = ident + 0*net: data-equal to ident but gates the PE
            # transposes behind the NET matmuls so the scheduler cannot
            # hoist them ahead; bf16 out so the transposes run 1-pass
            z0 = main.tile([P, 1], f32)
            nc.vector.tensor_scalar(
                out=z0, in0=netsrc[:, 0:1], scalar1=0.0, scalar2=None, op0=A.mult
            )
            identC = main.tile([P, V], bf16)
            nc.vector.tensor_scalar(
                out=identC, in0=ident, scalar1=z0[:, 0:1], scalar2=None, op0=A.add
            )
            net3 = view3(netsrc, 2, V)
            mx2 = main.tile([P, 2], f32)
            nc.vector.reduce_max(mx2, net3, axis=X)
            ge2 = main.tile([P, 2, V], bf16)
            nc.vector.tensor_tensor(
                out=ge2, in0=net3, in1=bcast_last(mx2, V), op=A.is_ge
            )
            ix2 = main.tile([P, 2, V], bf16)
            nc.vector.tensor_tensor(
                out=ix2, in0=ge2, in1=bcast_mid(iotb, 2), op=A.mult
            )
            LS = main.tile([P, 2], f32)
            nc.vector.reduce_max(LS, ix2, axis=X)
            # pkb = [L | S + 128*[S==0]]  (both <= 255, exact in bf16)
            zinv = main.tile([P, 1], f32)
            nc.vector.tensor_scalar(
                out=zinv, in0=LS[:, 1:2], scalar1=0.5, scalar2=None, op0=A.is_lt
            )
            pkb = main.tile([P, 2], bf16)
            nc.vector.tensor_copy(pkb[:, 0:1], LS[:, 0:1])
            nc.vector.tensor_scalar(
                out=pkb[:, 1:2], in0=zinv, scalar1=float(V), scalar2=LS[:, 1:2],
                op0=A.mult, op1=A.add,
            )

            # ---- PE transposes + ACT bf16 evac + PE [L|S'] gather ----
            xTs = []
            for h in range(NH):
                for j in range(HJ):
                    tp = ptp.tile([P, P], bf16, tag="tp", bufs=4)
                    nc.tensor.transpose(tp, xh[h][:, j, 0:V], identC)
                    xT = main.tile([P, P], bf16, tag=f"xT{h}{j}")
                    nc.scalar.activation(out=xT, in_=tp, func=AF.Copy)
                    xTs.append(xT)
            qhl = pq.tile([P, 2 * NJ], f32)
            for sj in range(NJ):
                nc.tensor.matmul(
                    qhl[:, 2 * sj : 2 * sj + 2], lhsT=xTs[sj], rhs=pkb,
                    start=True, stop=True,
                )

            # ---- DVE: unpack -> c = (L + a1*S) & 127 | dead-flag ----
            qi = main.tile([P, 2 * NJ], i32)
            nc.vector.tensor_copy(qi, qhl)
            lov = stridev(qi, 0, 2, NJ)
            hiv = stridev(qi, 1, 2, NJ)
            si = main.tile([P, NJ], i32)
            nc.vector.tensor_scalar(out=si, in0=hiv, scalar1=V - 1, scalar2=None, op0=A.bitwise_and)
            di = main.tile([P, NJ], i32)
            nc.vector.tensor_scalar(out=di, in0=hiv, scalar1=V, scalar2=None, op0=A.bitwise_and)
            ti = main.tile([P, NJ], i32)
            nc.vector.tensor_mul(ti, si, a1i)
            nc.vector.tensor_tensor(out=ti, in0=ti, in1=lov, op=A.add)
            ci = main.tile([P, NJ], i32)
            nc.vector.tensor_scalar(out=ci, in0=ti, scalar1=V - 1, scalar2=None, op0=A.bitwise_and)
            nc.vector.tensor_tensor(out=ci, in0=ci, in1=di, op=A.bitwise_or)
            cfb = main.tile([P, NJ], bf16)
            nc.vector.tensor_copy(cfb, ci)

            # ---- one fused z1 build; halves stream on sync+scalar ----
            zt = main.tile([P, NJ, V], f32)
            nc.vector.tensor_tensor(
                out=zt,
                in0=bcast_mid(iotb, NJ),
                in1=bcast_last(cfb, V),
                op=A.is_equal,
            )
            zt3 = zt  # [P, NJ, V]
            nc.sync.dma_start(
                out=out_r[:, 0:HJ, V : 2 * V], in_=zt3[:, 0:HJ, :]
            )
            nc.scalar.dma_start(
                out=out_r[:, HJ:NJ, V : 2 * V], in_=zt3[:, HJ:NJ, :]
            )

    nc.finalize()
    return nc


def _pack_weights(W1, b1, W2, b2, has_b1, has_b2) -> np.ndarray:
    """Pure layout marshalling of the MLP weights into one [P, wpk] block."""
    wpk = WPK + (KH if has_b1 else 0) + (2 * V if has_b2 else 0)
    wpack = np.empty((P, wpk), np.float32)
    # w1t[p, k*V+i] = W1[i, k*P+p]
    wpack[:, W1T_OFF:W2_OFF] = (
        W1.T.reshape(KH, P, V).transpose(1, 0, 2).reshape(P, KH * V)
    )
    # w2p[p, k*2V+n] = W2[k*P+p, n]
    wpack[:, W2_OFF:IOTA_OFF] = (
        W2.reshape(KH, P, 2 * V).transpose(1, 0, 2).reshape(P, KH * 2 * V)
    )
    wpack[:, IOTA_OFF:IDENT_OFF] = np.arange(V, dtype=np.float32)
    wpack[:, IDENT_OFF:WPK] = np.eye(V, dtype=np.float32)
    off = WPK
    if has_b1:
        wpack[:, off : off + KH] = b1.reshape(KH, P).T
        off += KH
    if has_b2:
        wpack[:, off : off + 2 * V] = b2.reshape(1, 2 * V)
    return wpack


# Test-harness hooks: extra kwargs for run_bass_kernel_spmd (e.g. trace=True)
# and the last BassKernelResults for profiling. Unused when graded.
RUN_KWARGS: dict = {}
LAST_RESULTS = None


def kernel(**inputs) -> np.ndarray:
    global LAST_RESULTS
    import ml_dtypes
    from concourse.bass_utils import run_bass_kernel_spmd

    # lossless cast: inputs are exact one-hots (0.0/1.0), both exact in bf16
    x = np.ascontiguousarray(
        np.asarray(inputs["inputs"], dtype=np.float32).astype(ml_dtypes.bfloat16)
    )
    W1 = np.asarray(inputs["W1"], dtype=np.float32)
    b1 = np.asarray(inputs["b1"], dtype=np.float32)
    W2 = np.asarray(inputs["W2"], dtype=np.float32)
    b2 = np.asarray(inputs["b2"], dtype=np.float32)
    has_b1 = bool(np.any(b1))
    has_b2 = bool(np.any(b2))
    wpack = _pack_weights(W1, b1, W2, b2, has_b1, has_b2)

    B = x.shape[0]
    rows = B // N_CORES
    nc = build_bass(rows, has_b1, has_b2)

    shards = np.split(x, N_CORES, axis=0)
    in_maps = [{"x": s, "wpack": wpack} for s in shards]
    res = run_bass_kernel_spmd(nc, in_maps, list(range(N_CORES)), **RUN_KWARGS)
    LAST_RESULTS = res
    return np.concatenate([r["out"] for r in res.results], axis=0)
